# revision 19
# baseline (speedup 1.0000x reference)
"""Trainium2 Bass kernel for nn_FCond (FiLM-conditioned MLP chain).

Reference computation (B=32, N=100000, D=3, CDIM=128):
    h = x
    for kblk in [0, 1, 2, 2, 2, 2]:
        h = tanh((h @ Wk.T + bk) * sigmoid(c @ Wsk.T + bsk) + (c @ Wbk.T + bbk))

Since the FiLM conditioning depends only on (c, weights), each (batch,
block) reduces to an affine map  h' = tanh(A_kb @ h + d_kb)  with
A_kb [3,3], d_kb [3] precomputed on the host in float64.

Device strategy (pure data parallel over 8 cores, 4 batches/core):
  - Layout: partition p = b*30 + comp*10 + g (4 batch-bands of 30 rows,
    partitions 0..119). Partition 120 is a persistent ones-row: the
    per-batch bias d is folded into the matmul (W[120, out] = d); the
    ones are DMA'd once into every h buffer at startup and no engine
    ever overwrites them (tanh writes rows [0:120] only; matmuls
    contract over [0:121]).
  - Dtypes: x in f16, y out f16. Intermediate h tiles are f32, bitcast
    to float32r for the matmul moving operand (1 col/cycle, and avoids
    the ~1.25x ScalarE f16-store penalty). Weights f32r via one ACT
    copy; block 0 uses a separate f16 W0 so its moving operand (f16 x)
    matches.
  - tanh runs on TWO engines concurrently:
      * ScalarE (ACT): native Tanh, PSUM->SBUF, ~1 col/cyc @ 1.2 GHz.
      * VectorE (DVE): runtime-registered custom op TANH7_ANT -- the
        degree-7 odd minimax polynomial z*(d0 + d1 t + d2 t^2 + d3 t^3),
        t = z^2, one fused 8-stage pass, PSUM->SBUF, ~1 col/cyc
        @ 0.96 GHz. Fit on |z| <= 2.1 (max err 3.7e-3); blocks 1-5 have
        measured |z| <= 1.87 so no clamp is needed. Block 0 (|z| <= 3.6)
        always uses ACT. All block-5 (output) units go to DVE, whose f16
        stores are full-rate.
  - Work is split into 60 (chunk, block) units of <=1024 columns
    (10 chunks: 784 + 9x1024). PSUM holds 4 independent 1024-col f32
    regions, so each consumer engine is double-buffered and PE can run
    ~2 units ahead. Units are emitted in diagonal wave order
    (key = chunk + 1.8*block), which keeps consecutive blocks of a
    chunk ~5 units apart and spreads chunk-DMA arrivals.
  - Measured end-to-end rel err ~5e-3 (tolerance 2e-2).
"""
import sys
import types

import numpy as np

B, N, D, CDIM = 32, 100000, 3, 128
NCORES = 8
BPC = B // NCORES          # batches per core
G = 10                     # point-groups per (batch, comp)
L = 10000                  # points per partition stream (N / G, exact)
P = 128                    # partitions
NROWS = 120                # data rows (4 bands x 30)
ONES_ROW = 120             # persistent ones partition
KDIM = 121                 # matmul contraction/out rows [0:121]
MM_F = 512                 # matmul free chunk (1 PSUM bank)
NCHUNK = 10
UNIT = 1024                # psum region / consumer instruction width
NHBUF = 12                 # h tile ring size

# degree-7 odd minimax tanh coefficients, fit on |z| <= 2.1 (err 3.7e-3)
D0 = 0.98778855
D1 = -0.27433429
D2 = 0.05638376
D3 = -0.00482919

SIZES = (784,) + (1024,) * 9
OFFS = tuple(np.cumsum((0,) + SIZES[:-1]).tolist())
WSETS = (0, 1, 2, 2, 2, 2)

# 60 (chunk, block) units in diagonal wave order
UNIT_ORDER = sorted(((c, k) for c in range(NCHUNK) for k in range(6)),
                    key=lambda u: (u[0] + 1.5 * u[1], u[1]))
# DVE units: all of block 5, plus (c+k) odd on blocks 1-4 minus 2 flips
# to balance engine load (ACT ~33k cols @1.2G, DVE ~27k cols @0.96G);
# (9,4) flipped so the final DVE chain (9,4)->(9,5) splits across engines
_FLIPS = {(2, 3), (9, 4)}
DVE_UNITS = ({(c, 5) for c in range(NCHUNK)} |
             {(c, k) for c in range(NCHUNK) for k in range(1, 5)
              if (c + k) % 2 == 1} - _FLIPS)

PROFILE = False            # set by test harness; collects HW exec time
LAST_EXEC_NS = None

_CACHE = {}


def _install_profile_shim():
    """Register the NTFF profile hook (missing antenv.axon_hooks in this
    container) so run_bass_kernel_spmd(trace=True) can report exec time."""
    if "antenv.axon_hooks" in sys.modules:
        return
    mod = types.ModuleType("antenv.axon_hooks")
    _state = {"hook": None}
    mod.set_axon_ntff_profile_hook = lambda h: _state.__setitem__("hook", h)
    mod.get_axon_ntff_profile_hook = lambda: _state["hook"]
    sys.modules["antenv.axon_hooks"] = mod
    try:
        from trn_agent_boot.trn_boot import _ntff_profile_via_ctypes
        mod.set_axon_ntff_profile_hook(
            _ntff_profile_via_ctypes("/opt/axon/libaxon_pjrt.so"))
    except Exception:
        pass
    import concourse.bass_utils as bu
    bu.upload_artifacts = lambda tmpdir: f"local:{tmpdir}"


def _register_tanh7():
    """Register the TANH7_ANT custom DVE op (degree-7 odd Horner + final
    multiply, 8 ALU stages, one streaming pass) via the documented
    dve_ops authoring API. Coefficients d3,d2,d1 ride the s0/s1/imm2
    scalar slots; d0 is the C3 spill delivered via in1."""
    from concourse import dve_ops
    for o in dve_ops.OPS:
        if o.name == "TANH7_ANT":
            return o
    from concourse.dve_spec import (
        C0, C1, C2, C3, Spec, Src0, _spill_c3_to_src1, lower, sq,
    )
    from concourse.dve_uop import DveOpSpec

    t = sq(Src0)
    body = _spill_c3_to_src1((((C0 * t + C1) * t + C2) * t + C3) * Src0)

    def ref(in0, in1, s0, s1, imm2):
        x = np.asarray(in0, np.float32)
        tt = x * x
        d0 = np.asarray(in1, np.float32)[:, :1]
        return (((s0 * tt + s1) * tt + imm2) * tt + d0) * x

    spec = Spec(body=body, reference=ref)
    row = max(dve_ops._SUB_OPCODE_FOR_NAME.values()) + 1
    shas = {
        ver: DveOpSpec(name="TANH7_ANT", opcode=row,
                       uops=lower(spec, ver=ver), rd1_en=True).sha(ver)
        for ver in ("v3", "v4")
    }
    op = dve_ops.DveOp("TANH7_ANT", spec, subdim=False, uops_sha=shas)
    dve_ops.OPS.append(op)
    dve_ops.CUSTOM_DVE_SPECS[op.name] = spec
    dve_ops._SUB_OPCODE_FOR_NAME[op.name] = row
    return op


def _build_program():
    import concourse.bacc as bacc
    import concourse.tile as tile
    from concourse import mybir

    f32 = mybir.dt.float32
    f32r = mybir.dt.float32r
    f16 = mybir.dt.float16
    Tanh = mybir.ActivationFunctionType.Tanh
    Copy = mybir.ActivationFunctionType.Copy
    tanh7 = _register_tanh7()

    nc = bacc.Bacc("TRN2", target_bir_lowering=False, debug=False)
    x_d = nc.declare_dram_parameter("x", [P, L], f16, isOutput=False)
    w_d = nc.declare_dram_parameter("w", [3, P, P], f32, isOutput=False)
    w016_d = nc.declare_dram_parameter("w016", [P, P], f16, isOutput=False)
    ones_d = nc.declare_dram_parameter("ones", [1, UNIT], f32,
                                       isOutput=False)
    y_d = nc.declare_dram_parameter("y", [P, L], f16, isOutput=True)

    with tile.TileContext(nc) as tc:
        with (
            tc.tile_pool(name="wpool", bufs=1) as wpool,
            tc.tile_pool(name="xinpool", bufs=NCHUNK) as xinpool,
            tc.tile_pool(name="youtpool", bufs=NCHUNK) as youtpool,
            tc.tile_pool(name="hpool", bufs=NHBUF) as hpool,
            tc.tile_pool(name="psum", bufs=4, space="PSUM") as psum,
        ):
            # --- first compute chunk + block-0 weights go out first so
            # the PE/ACT chain can start ASAP ---
            h0 = xinpool.tile([P, SIZES[0]], f16, name="xin0", tag="xin")
            nc.sync.dma_start(h0[:], x_d[:, 0:SIZES[0]])
            w016 = wpool.tile([P, P], f16, name="w016", tag="w016")
            nc.sync.dma_start(w016[:], w016_d[:])
            wraw = wpool.tile([P, 3 * P], f32, name="wraw", tag="wraw")
            for k in range(3):
                nc.sync.dma_start(wraw[:, k * P:(k + 1) * P], w_d[k])

            # remaining input chunks on the sync queue
            hs = {0: h0}
            for ci in range(1, NCHUNK):
                h = xinpool.tile([P, SIZES[ci]], f16, name=f"xin{ci}",
                                 tag="xin")
                nc.sync.dma_start(
                    h[:], x_d[:, OFFS[ci]:OFFS[ci] + SIZES[ci]])
                hs[ci] = h

            # weights: one ACT copy rounds f32 -> f32r (the Tanh table
            # load is auto-inserted by Bacc.insert_act_table_loads)
            wall = wpool.tile([P, 3 * P], f32r, name="wall", tag="wall")
            nc.scalar.activation(wall[:], wraw[:], Copy)
            wts = [wall[0:KDIM, k * P:k * P + KDIM] for k in range(3)]
            wt0 = w016[0:KDIM, 0:KDIM]

            # d0 coefficient vector for the custom op's C3/in1 latch
            d0c = wpool.tile([P, 1], f32, name="d0c", tag="d0c")
            nc.vector.memset(d0c[:], D0)

            # PE warmup with NO data dependencies (memset-fed) so it runs
            # the moment the engines start, ramping the HAM clock while
            # the input DMAs are still in flight; targets unit (0,0)'s
            # psum region, which the real matmuls overwrite (start=True)
            warm = wpool.tile([P, MM_F], f32, name="warm", tag="warm")
            nc.vector.memset(warm[:], 0.0)
            warm_ps = psum.tile([P, SIZES[0]], f32, name="ps0_0", tag="ps")
            for _ in range(2):
                nc.tensor.matmul(warm_ps[0:KDIM, 0:MM_F], warm[:, 0:KDIM],
                                 warm[:], start=True, stop=True)

            ps_tiles = {}

            def emit_matmuls(ci, kblk):
                sz = SIZES[ci]
                if (ci, kblk) == (0, 0):
                    ps = warm_ps       # pre-allocated (warmup target)
                else:
                    ps = psum.tile([P, sz], f32, name=f"ps{ci}_{kblk}",
                                   tag="ps")
                ps_tiles[(ci, kblk)] = ps
                if kblk == 0:
                    rhs, w = hs[ci], wt0
                else:
                    rhs, w = hs[ci], wts[WSETS[kblk]]
                for j in range(0, sz, MM_F):
                    je = min(j + MM_F, sz)
                    nc.tensor.matmul(ps[0:KDIM, j:je], w,
                                     rhs[0:KDIM, j:je],
                                     start=True, stop=True)

            def emit_act(ci, kblk):
                ps = ps_tiles[(ci, kblk)]
                hn = hpool.tile([P, SIZES[ci]], f32r,
                                name=f"h{ci}_{kblk}", tag="h")
                # rows [0:121]: row 120 self-regenerates the ones row
                # (psum row 120 = W[120,120]*1 = 9.5, tanh -> 1.0)
                nc.scalar.activation(hn[0:KDIM, :], ps[0:KDIM, :],
                                     Tanh, bias=0.0, scale=1.0)
                hs[ci] = hn

            def emit_dve(ci, kblk):
                sz = SIZES[ci]
                ps = ps_tiles[(ci, kblk)]
                if kblk == 5:
                    yo = youtpool.tile([P, sz], f16, name=f"yo{ci}",
                                       tag="yout")
                    c0 = OFFS[ci]
                    # split the last two chunks' output so the final DMA
                    # overlaps the final tanh
                    parts = ((0, sz // 2, sz) if ci >= NCHUNK - 2
                             else (0, sz))
                    for a0, b0 in zip(parts[:-1], parts[1:]):
                        nc.vector._custom_dve(
                            tanh7, out=yo[0:NROWS, a0:b0],
                            in0=ps[0:NROWS, a0:b0],
                            in1=d0c[0:NROWS, :],
                            s0=D3, s1=D2, imm2=D1)
                        nc.sync.dma_start(y_d[0:NROWS, c0 + a0:c0 + b0],
                                          yo[0:NROWS, a0:b0])
                    return
                hn = hpool.tile([P, sz], f32r, name=f"h{ci}_{kblk}",
                                tag="h")
                nc.vector._custom_dve(
                    tanh7, out=hn[0:NROWS, :], in0=ps[0:NROWS, :],
                    in1=d0c[0:NROWS, :], s0=D3, s1=D2, imm2=D1)
                # restore the ones row the polynomial can't produce
                nc.sync.dma_start(hn[ONES_ROW:ONES_ROW + 1, 0:sz],
                                  ones_d[:, 0:sz].bitcast(f32r))
                hs[ci] = hn

            for (ci, kblk) in UNIT_ORDER:
                emit_matmuls(ci, kblk)
                if (ci, kblk) in DVE_UNITS:
                    emit_dve(ci, kblk)
                else:
                    emit_act(ci, kblk)
    nc.compile()
    return nc


def _film_params(c, Wk, bk, Wsk, bsk, Wbk, bbk):
    """A[b] = diag(scale[b]) @ Wk ; d[b] = scale[b]*bk + shift[b], float64."""
    c = c.astype(np.float64)
    scale = 1.0 / (1.0 + np.exp(-(c @ Wsk.astype(np.float64).T
                                  + bsk.astype(np.float64))))     # [B,3]
    shift = c @ Wbk.astype(np.float64).T + bbk.astype(np.float64)  # [B,3]
    A = scale[:, :, None] * Wk.astype(np.float64)[None]            # [B,3,3]
    d = scale * bk.astype(np.float64) + shift                      # [B,3]
    return A, d


def kernel(t, x, c,
           W0, b0, Ws0, bs0, Wb0, bb0,
           W1, b1, Ws1, bs1, Wb1, bb1,
           W2, b2, Ws2, bs2, Wb2, bb2):
    global LAST_EXEC_NS
    if PROFILE:
        _install_profile_shim()
    from concourse.bass_utils import run_bass_kernel_spmd

    x = np.asarray(x)
    c = np.asarray(c)
    (W0, b0, Ws0, bs0, Wb0, bb0, W1, b1, Ws1, bs1, Wb1, bb1,
     W2, b2, Ws2, bs2, Wb2, bb2) = (
        np.asarray(a) for a in (W0, b0, Ws0, bs0, Wb0, bb0,
                                W1, b1, Ws1, bs1, Wb1, bb1,
                                W2, b2, Ws2, bs2, Wb2, bb2))
    out_dtype = x.dtype

    if "prog" not in _CACHE:
        _CACHE["prog"] = _build_program()
    nc = _CACHE["prog"]

    # ---- host: FiLM affine params per (weight-set, batch), float64 ----
    sets = [
        _film_params(c, W0, b0, Ws0, bs0, Wb0, bb0),
        _film_params(c, W1, b1, Ws1, bs1, Wb1, bb1),
        _film_params(c, W2, b2, Ws2, bs2, Wb2, bb2),
    ]

    # ---- host: shard + relayout x (f16) ----
    # [B, N, 3] -> per core [128, L]: p = b*30 + comp*10 + g; row 120 = 1
    xt = np.ascontiguousarray(
        np.asarray(x, np.float32).transpose(0, 2, 1)
    ).reshape(B, D, G, L).astype(np.float16)
    ones = np.ones((1, UNIT), np.float32)

    in_maps = []
    for cc in range(NCORES):
        bs = range(cc * BPC, (cc + 1) * BPC)
        X = np.zeros((P, L), np.float16)
        X[:NROWS] = xt[cc * BPC:(cc + 1) * BPC].reshape(NROWS, L)
        X[ONES_ROW] = np.float16(1.0)
        W6 = np.zeros((3, P, P), np.float32)
        for k in range(3):
            A, dv = sets[k]
            A32 = A.astype(np.float32)
            d32 = dv.astype(np.float32)
            for i, b in enumerate(bs):
                for ci_ in range(3):
                    o = i * 30 + ci_ * G
                    for cj in range(3):
                        a = A32[b, ci_, cj]
                        for g in range(G):
                            W6[k, i * 30 + cj * G + g, o + g] = a
                    # folded bias: ones-row (120) -> output rows of comp ci_
                    W6[k, ONES_ROW, o:o + G] = d32[b, ci_]
            W6[k, ONES_ROW, ONES_ROW] = 9.5   # ones self-regen via tanh
        in_maps.append({"x": X, "w": W6,
                        "w016": W6[0].astype(np.float16), "ones": ones})

    res = run_bass_kernel_spmd(nc, in_maps, list(range(NCORES)),
                               trace=bool(PROFILE))
    if PROFILE:
        LAST_EXEC_NS = res.exec_time_ns

    # ---- host: gather + inverse layout ----
    out = np.empty((B, N, D), out_dtype)
    for cc in range(NCORES):
        Y = res.results[cc]["y"][:NROWS].reshape(BPC, D, N)
        for i in range(BPC):
            out[cc * BPC + i] = Y[i].T.astype(np.float32)
    return out


# revision 20
# speedup vs baseline: 1.1181x; 1.1181x over previous
"""Trainium2 Bass kernel for nn_FCond (FiLM-conditioned MLP chain).

Reference computation (B=32, N=100000, D=3, CDIM=128):
    h = x
    for kblk in [0, 1, 2, 2, 2, 2]:
        h = tanh((h @ Wk.T + bk) * sigmoid(c @ Wsk.T + bsk) + (c @ Wbk.T + bbk))

Since the FiLM conditioning depends only on (c, weights), each (batch,
block) reduces to an affine map  h' = tanh(A_kb @ h + d_kb)  with
A_kb [3,3], d_kb [3] precomputed on the host in float64.

Device strategy (pure data parallel over 8 cores, 4 batches/core):
  - Layout: partition p = b*30 + comp*10 + g (4 batch-bands of 30 rows,
    partitions 0..119). Partition 120 is a persistent ones-row: the
    per-batch bias d is folded into the matmul (W[120, out] = d); the
    ones are DMA'd once into every h buffer at startup and no engine
    ever overwrites them (tanh writes rows [0:120] only; matmuls
    contract over [0:121]).
  - Dtypes: x in f16, y out f16. Intermediate h tiles are f32, bitcast
    to float32r for the matmul moving operand (1 col/cycle, and avoids
    the ~1.25x ScalarE f16-store penalty). Weights f32r via one ACT
    copy; block 0 uses a separate f16 W0 so its moving operand (f16 x)
    matches.
  - tanh runs on TWO engines concurrently:
      * ScalarE (ACT): native Tanh, PSUM->SBUF, ~1 col/cyc @ 1.2 GHz.
      * VectorE (DVE): runtime-registered custom op TANH7_ANT -- the
        degree-7 odd minimax polynomial z*(d0 + d1 t + d2 t^2 + d3 t^3),
        t = z^2, one fused 8-stage pass, PSUM->SBUF, ~1 col/cyc
        @ 0.96 GHz. Fit on |z| <= 2.1 (max err 3.7e-3); blocks 1-5 have
        measured |z| <= 1.87 so no clamp is needed. Block 0 (|z| <= 3.6)
        always uses ACT. All block-5 (output) units go to DVE, whose f16
        stores are full-rate.
  - Work is split into 60 (chunk, block) units of <=1024 columns
    (10 chunks: 784 + 9x1024). PSUM holds 4 independent 1024-col f32
    regions, so each consumer engine is double-buffered and PE can run
    ~2 units ahead. Units are emitted in diagonal wave order
    (key = chunk + 1.8*block), which keeps consecutive blocks of a
    chunk ~5 units apart and spreads chunk-DMA arrivals.
  - Measured end-to-end rel err ~5e-3 (tolerance 2e-2).
"""
import sys
import types

import numpy as np

B, N, D, CDIM = 32, 100000, 3, 128
NCORES = 8
BPC = B // NCORES          # batches per core
G = 10                     # point-groups per (batch, comp)
L = 10000                  # points per partition stream (N / G, exact)
P = 128                    # partitions
NROWS = 120                # data rows (4 bands x 30)
ONES_ROW = 120             # persistent ones partition
KDIM = 121                 # matmul contraction/out rows [0:121]
MM_F = 512                 # matmul free chunk (1 PSUM bank)
NCHUNK = 10
UNIT = 1024                # psum region / consumer instruction width
NHBUF = 12                 # h tile ring size

# degree-7 odd minimax tanh coefficients, fit on |z| <= 2.1 (err 3.7e-3)
D0 = 0.98778855
D1 = -0.27433429
D2 = 0.05638376
D3 = -0.00482919

SIZES = (784,) + (1024,) * 9
OFFS = tuple(np.cumsum((0,) + SIZES[:-1]).tolist())
WSETS = (0, 1, 2, 2, 2, 2)

# 60 (chunk, block) units in diagonal wave order
UNIT_ORDER = sorted(((c, k) for c in range(NCHUNK) for k in range(6)),
                    key=lambda u: (u[0] + 1.8 * u[1], u[1]))
# DVE units: all of block 5, plus (c+k) odd on blocks 1-4 minus 4 flips
# to balance engine load (ACT ~34k cols @1.2G, DVE ~26k cols @0.96G);
# flips sit mid/late so DVE still starts early; (9,4) flipped so the
# final chain (9,4)->(9,5) splits across both engines
_FLIPS = {(4, 3), (6, 3), (8, 1), (9, 4)}
DVE_UNITS = ({(c, 5) for c in range(NCHUNK)} |
             {(c, k) for c in range(NCHUNK) for k in range(1, 5)
              if (c + k) % 2 == 1} - _FLIPS)

PROFILE = False            # set by test harness; collects HW exec time
LAST_EXEC_NS = None

_CACHE = {}


def _install_profile_shim():
    """Register the NTFF profile hook (missing antenv.axon_hooks in this
    container) so run_bass_kernel_spmd(trace=True) can report exec time."""
    if "antenv.axon_hooks" in sys.modules:
        return
    mod = types.ModuleType("antenv.axon_hooks")
    _state = {"hook": None}
    mod.set_axon_ntff_profile_hook = lambda h: _state.__setitem__("hook", h)
    mod.get_axon_ntff_profile_hook = lambda: _state["hook"]
    sys.modules["antenv.axon_hooks"] = mod
    try:
        from trn_agent_boot.trn_boot import _ntff_profile_via_ctypes
        mod.set_axon_ntff_profile_hook(
            _ntff_profile_via_ctypes("/opt/axon/libaxon_pjrt.so"))
    except Exception:
        pass
    import concourse.bass_utils as bu
    bu.upload_artifacts = lambda tmpdir: f"local:{tmpdir}"


def _register_tanh7():
    """Register the TANH7_ANT custom DVE op (degree-7 odd Horner + final
    multiply, 8 ALU stages, one streaming pass) via the documented
    dve_ops authoring API. Coefficients d3,d2,d1 ride the s0/s1/imm2
    scalar slots; d0 is the C3 spill delivered via in1."""
    from concourse import dve_ops
    for o in dve_ops.OPS:
        if o.name == "TANH7_ANT":
            return o
    from concourse.dve_spec import (
        C0, C1, C2, C3, Spec, Src0, _spill_c3_to_src1, lower, sq,
    )
    from concourse.dve_uop import DveOpSpec

    t = sq(Src0)
    body = _spill_c3_to_src1((((C0 * t + C1) * t + C2) * t + C3) * Src0)

    def ref(in0, in1, s0, s1, imm2):
        x = np.asarray(in0, np.float32)
        tt = x * x
        d0 = np.asarray(in1, np.float32)[:, :1]
        return (((s0 * tt + s1) * tt + imm2) * tt + d0) * x

    spec = Spec(body=body, reference=ref)
    row = max(dve_ops._SUB_OPCODE_FOR_NAME.values()) + 1
    shas = {
        ver: DveOpSpec(name="TANH7_ANT", opcode=row,
                       uops=lower(spec, ver=ver), rd1_en=True).sha(ver)
        for ver in ("v3", "v4")
    }
    op = dve_ops.DveOp("TANH7_ANT", spec, subdim=False, uops_sha=shas)
    dve_ops.OPS.append(op)
    dve_ops.CUSTOM_DVE_SPECS[op.name] = spec
    dve_ops._SUB_OPCODE_FOR_NAME[op.name] = row
    return op


def _build_program():
    import concourse.bacc as bacc
    import concourse.tile as tile
    from concourse import mybir

    f32 = mybir.dt.float32
    f32r = mybir.dt.float32r
    f16 = mybir.dt.float16
    Tanh = mybir.ActivationFunctionType.Tanh
    Copy = mybir.ActivationFunctionType.Copy
    tanh7 = _register_tanh7()

    nc = bacc.Bacc("TRN2", target_bir_lowering=False, debug=False)
    x_d = nc.declare_dram_parameter("x", [P, L], f16, isOutput=False)
    w_d = nc.declare_dram_parameter("w", [3, P, P], f32, isOutput=False)
    w016_d = nc.declare_dram_parameter("w016", [P, P], f16, isOutput=False)
    ones_d = nc.declare_dram_parameter("ones", [1, UNIT], f32,
                                       isOutput=False)
    y_d = nc.declare_dram_parameter("y", [P, L], f16, isOutput=True)

    with tile.TileContext(nc) as tc:
        with (
            tc.tile_pool(name="wpool", bufs=1) as wpool,
            tc.tile_pool(name="xinpool", bufs=NCHUNK) as xinpool,
            tc.tile_pool(name="youtpool", bufs=NCHUNK) as youtpool,
            tc.tile_pool(name="hpool", bufs=NHBUF) as hpool,
            tc.tile_pool(name="psum", bufs=4, space="PSUM") as psum,
        ):
            # --- first compute chunk + block-0 weights go out first so
            # the PE/ACT chain can start ASAP ---
            h0 = xinpool.tile([P, SIZES[0]], f16, name="xin0", tag="xin")
            nc.sync.dma_start(h0[:], x_d[:, 0:SIZES[0]])
            w016 = wpool.tile([P, P], f16, name="w016", tag="w016")
            nc.sync.dma_start(w016[:], w016_d[:])
            wraw = wpool.tile([P, 3 * P], f32, name="wraw", tag="wraw")
            for k in range(3):
                nc.sync.dma_start(wraw[:, k * P:(k + 1) * P], w_d[k])

            # remaining input chunks on the sync queue
            hs = {0: h0}
            for ci in range(1, NCHUNK):
                h = xinpool.tile([P, SIZES[ci]], f16, name=f"xin{ci}",
                                 tag="xin")
                nc.sync.dma_start(
                    h[:], x_d[:, OFFS[ci]:OFFS[ci] + SIZES[ci]])
                hs[ci] = h

            # weights: one ACT copy rounds f32 -> f32r (the Tanh table
            # load is auto-inserted by Bacc.insert_act_table_loads)
            wall = wpool.tile([P, 3 * P], f32r, name="wall", tag="wall")
            nc.scalar.activation(wall[:], wraw[:], Copy)
            wts = [wall[0:KDIM, k * P:k * P + KDIM] for k in range(3)]
            wt0 = w016[0:KDIM, 0:KDIM]

            # d0 coefficient vector for the custom op's C3/in1 latch
            d0c = wpool.tile([P, 1], f32, name="d0c", tag="d0c")
            nc.vector.memset(d0c[:], D0)

            # PE warmup with NO data dependencies (memset-fed) so it runs
            # the moment the engines start, ramping the HAM clock while
            # the input DMAs are still in flight; targets unit (0,0)'s
            # psum region, which the real matmuls overwrite (start=True)
            warm = wpool.tile([P, MM_F], f32, name="warm", tag="warm")
            nc.vector.memset(warm[:], 0.0)
            warm_ps = psum.tile([P, SIZES[0]], f32, name="ps0_0", tag="ps")
            for _ in range(2):
                nc.tensor.matmul(warm_ps[0:KDIM, 0:MM_F], warm[:, 0:KDIM],
                                 warm[:], start=True, stop=True)

            ps_tiles = {}

            def emit_matmuls(ci, kblk):
                sz = SIZES[ci]
                if (ci, kblk) == (0, 0):
                    ps = warm_ps       # pre-allocated (warmup target)
                else:
                    ps = psum.tile([P, sz], f32, name=f"ps{ci}_{kblk}",
                                   tag="ps")
                ps_tiles[(ci, kblk)] = ps
                if kblk == 0:
                    rhs, w = hs[ci], wt0
                else:
                    rhs, w = hs[ci], wts[WSETS[kblk]]
                for j in range(0, sz, MM_F):
                    je = min(j + MM_F, sz)
                    nc.tensor.matmul(ps[0:KDIM, j:je], w,
                                     rhs[0:KDIM, j:je],
                                     start=True, stop=True)

            def emit_act(ci, kblk):
                ps = ps_tiles[(ci, kblk)]
                hn = hpool.tile([P, SIZES[ci]], f32r,
                                name=f"h{ci}_{kblk}", tag="h")
                # rows [0:121]: row 120 self-regenerates the ones row
                # (psum row 120 = W[120,120]*1 = 9.5, tanh -> 1.0)
                nc.scalar.activation(hn[0:KDIM, :], ps[0:KDIM, :],
                                     Tanh, bias=0.0, scale=1.0)
                hs[ci] = hn

            def emit_dve(ci, kblk):
                sz = SIZES[ci]
                ps = ps_tiles[(ci, kblk)]
                if kblk == 5:
                    yo = youtpool.tile([P, sz], f16, name=f"yo{ci}",
                                       tag="yout")
                    c0 = OFFS[ci]
                    # split the last two chunks' output so the final DMA
                    # overlaps the final tanh
                    parts = ((0, sz // 2, sz) if ci >= NCHUNK - 2
                             else (0, sz))
                    for a0, b0 in zip(parts[:-1], parts[1:]):
                        nc.vector._custom_dve(
                            tanh7, out=yo[0:NROWS, a0:b0],
                            in0=ps[0:NROWS, a0:b0],
                            in1=d0c[0:NROWS, :],
                            s0=D3, s1=D2, imm2=D1)
                        nc.sync.dma_start(y_d[0:NROWS, c0 + a0:c0 + b0],
                                          yo[0:NROWS, a0:b0])
                    return
                hn = hpool.tile([P, sz], f32r, name=f"h{ci}_{kblk}",
                                tag="h")
                nc.vector._custom_dve(
                    tanh7, out=hn[0:NROWS, :], in0=ps[0:NROWS, :],
                    in1=d0c[0:NROWS, :], s0=D3, s1=D2, imm2=D1)
                # restore the ones row the polynomial can't produce
                # (GPSIMD queue: keeps the sync DMA queue free for
                # input/output transfers, which issue in order)
                nc.gpsimd.dma_start(hn[ONES_ROW:ONES_ROW + 1, 0:sz],
                                    ones_d[:, 0:sz].bitcast(f32r))
                hs[ci] = hn

            for (ci, kblk) in UNIT_ORDER:
                emit_matmuls(ci, kblk)
                if (ci, kblk) in DVE_UNITS:
                    emit_dve(ci, kblk)
                else:
                    emit_act(ci, kblk)
    nc.compile()
    return nc


def _film_params(c, Wk, bk, Wsk, bsk, Wbk, bbk):
    """A[b] = diag(scale[b]) @ Wk ; d[b] = scale[b]*bk + shift[b], float64."""
    c = c.astype(np.float64)
    scale = 1.0 / (1.0 + np.exp(-(c @ Wsk.astype(np.float64).T
                                  + bsk.astype(np.float64))))     # [B,3]
    shift = c @ Wbk.astype(np.float64).T + bbk.astype(np.float64)  # [B,3]
    A = scale[:, :, None] * Wk.astype(np.float64)[None]            # [B,3,3]
    d = scale * bk.astype(np.float64) + shift                      # [B,3]
    return A, d


def kernel(t, x, c,
           W0, b0, Ws0, bs0, Wb0, bb0,
           W1, b1, Ws1, bs1, Wb1, bb1,
           W2, b2, Ws2, bs2, Wb2, bb2):
    global LAST_EXEC_NS
    if PROFILE:
        _install_profile_shim()
    from concourse.bass_utils import run_bass_kernel_spmd

    x = np.asarray(x)
    c = np.asarray(c)
    (W0, b0, Ws0, bs0, Wb0, bb0, W1, b1, Ws1, bs1, Wb1, bb1,
     W2, b2, Ws2, bs2, Wb2, bb2) = (
        np.asarray(a) for a in (W0, b0, Ws0, bs0, Wb0, bb0,
                                W1, b1, Ws1, bs1, Wb1, bb1,
                                W2, b2, Ws2, bs2, Wb2, bb2))
    out_dtype = x.dtype

    if "prog" not in _CACHE:
        _CACHE["prog"] = _build_program()
    nc = _CACHE["prog"]

    # ---- host: FiLM affine params per (weight-set, batch), float64 ----
    sets = [
        _film_params(c, W0, b0, Ws0, bs0, Wb0, bb0),
        _film_params(c, W1, b1, Ws1, bs1, Wb1, bb1),
        _film_params(c, W2, b2, Ws2, bs2, Wb2, bb2),
    ]

    # ---- host: shard + relayout x (f16) ----
    # [B, N, 3] -> per core [128, L]: p = b*30 + comp*10 + g; row 120 = 1
    xt = np.ascontiguousarray(
        np.asarray(x, np.float32).transpose(0, 2, 1)
    ).reshape(B, D, G, L).astype(np.float16)
    ones = np.ones((1, UNIT), np.float32)

    in_maps = []
    for cc in range(NCORES):
        bs = range(cc * BPC, (cc + 1) * BPC)
        X = np.zeros((P, L), np.float16)
        X[:NROWS] = xt[cc * BPC:(cc + 1) * BPC].reshape(NROWS, L)
        X[ONES_ROW] = np.float16(1.0)
        W6 = np.zeros((3, P, P), np.float32)
        for k in range(3):
            A, dv = sets[k]
            A32 = A.astype(np.float32)
            d32 = dv.astype(np.float32)
            for i, b in enumerate(bs):
                for ci_ in range(3):
                    o = i * 30 + ci_ * G
                    for cj in range(3):
                        a = A32[b, ci_, cj]
                        for g in range(G):
                            W6[k, i * 30 + cj * G + g, o + g] = a
                    # folded bias: ones-row (120) -> output rows of comp ci_
                    W6[k, ONES_ROW, o:o + G] = d32[b, ci_]
            W6[k, ONES_ROW, ONES_ROW] = 9.5   # ones self-regen via tanh
        in_maps.append({"x": X, "w": W6,
                        "w016": W6[0].astype(np.float16), "ones": ones})

    res = run_bass_kernel_spmd(nc, in_maps, list(range(NCORES)),
                               trace=bool(PROFILE))
    if PROFILE:
        LAST_EXEC_NS = res.exec_time_ns

    # ---- host: gather + inverse layout ----
    out = np.empty((B, N, D), out_dtype)
    for cc in range(NCORES):
        Y = res.results[cc]["y"][:NROWS].reshape(BPC, D, N)
        for i in range(BPC):
            out[cc * BPC + i] = Y[i].T.astype(np.float32)
    return out
